# revision 7
# baseline (speedup 1.0000x reference)
"""DualRelGCN message-passing kernel for 8 TRN2 NeuronCores.

Strategy (destination-sharded, collective-free, block-dense):
  - LayerNorm is invariant to positive per-row scaling, so LN(agg/denom) ==
    LN(agg): the denominator drops out of the computation entirely.
  - Shard edges by dst range: core c owns nodes [1250c, 1250(c+1)) and
    receives every edge whose dst falls there.  Each core computes its 1250
    output rows completely locally -> no collectives.
  - The weighted gather+segment_sum is expressed as a block matmul:
    agg[tile t] = sum_s W_ts.T @ X_s, with dense fp8 adjacency blocks
    host-scattered from the edge list.
  - v2 layout: everything SBUF-resident (W 12.9 MB + X 2.5 MB fit easily),
    DMA issued in exact PE-consumption order on one ring at full bandwidth,
    no buffer recycling -> no PE->DMA feedback stalls.  relslice and the
    output travel as bf16 (host upcasts); the pad src tile is never stored.
"""

import sys

for _p in ("/opt/trn_rl_repo",):
    if _p not in sys.path:
        sys.path.insert(0, _p)

from contextlib import ExitStack

import numpy as np
import ml_dtypes

import concourse.bacc as bacc
import concourse.mybir as mybir
from concourse.alu_op_type import AluOpType
from concourse.tile import TileContext
from concourse.bass_utils import run_bass_kernel_spmd

F32 = mybir.dt.float32
BF16 = mybir.dt.bfloat16
FP8 = mybir.dt.float8e4
AF = mybir.ActivationFunctionType

N_NODES = 10000
DIM = 256
N_CORES = 8
NODES_PER_CORE = N_NODES // N_CORES  # 1250
TILE = 128
N_TILES = -(-NODES_PER_CORE // TILE)  # 10 dst tiles per core
S_TILES = -(-N_NODES // TILE)  # 79 src tiles (no pad tile stored)
S_PAIRS = S_TILES // 2  # 39 DoubleRow pairs + 1 single tile
OUT_ROWS = N_TILES * TILE  # 1280
ALPHA = 0.1
LN_EPS = 1e-5
# interleave boundaries (src-tile units) for the X / W0 startup stream
CHUNKS0 = [0, 4, 12, 28, 52, S_TILES]
# steady-state W tiles arrive in two chunks each
CHUNKS = [0, 40, S_TILES]
# the last tile's tail chunk is small so its matmuls finish right behind DMA
CHUNKS_LAST = [0, 40, 72, S_TILES]

_CACHE: dict = {}


def _build():
    nc = bacc.Bacc("TRN2", target_bir_lowering=False, debug=False,
                   num_devices=N_CORES)

    x_d = nc.dram_tensor("x", [128, S_TILES * DIM], FP8,
                         kind="ExternalInput")
    w_d = nc.dram_tensor("wblk", [N_TILES, 128, S_TILES * TILE], FP8,
                         kind="ExternalInput")
    relsl_d = nc.dram_tensor("relslice", [128, N_TILES * DIM], BF16,
                             kind="ExternalInput")
    pwt_d = nc.dram_tensor("projwT", [128, 2 * DIM], BF16,
                           kind="ExternalInput")
    out_d = nc.dram_tensor("out", [128, N_TILES * DIM], BF16,
                           kind="ExternalOutput")

    with TileContext(nc) as tc, ExitStack() as es:
        const_pool = es.enter_context(tc.tile_pool(name="const", bufs=1))
        ep_pool = es.enter_context(tc.tile_pool(name="ep", bufs=2))
        ps_agg = es.enter_context(tc.tile_pool(name="ps_agg", bufs=4,
                                               space="PSUM"))
        ps_tr = es.enter_context(tc.tile_pool(name="ps_tr", bufs=2,
                                              space="PSUM"))
        ps_y = es.enter_context(tc.tile_pool(name="ps_y", bufs=2,
                                             space="PSUM"))

        # --- constants / resident inputs (scalar ring; small) ---
        iota_row = const_pool.tile([128, 128], F32, tag="iota")
        nc.gpsimd.iota(iota_row[:], [[1, 128]], base=0, channel_multiplier=0,
                       allow_small_or_imprecise_dtypes=True)
        pidx = const_pool.tile([128, 1], F32, tag="pidx")
        nc.gpsimd.iota(pidx[:], [[1, 1]], base=0, channel_multiplier=1,
                       allow_small_or_imprecise_dtypes=True)
        ident = const_pool.tile([128, 128], BF16, tag="ident")
        nc.vector.tensor_scalar(ident[:], iota_row[:], pidx[:], None,
                                AluOpType.is_equal)
        epsb = const_pool.tile([128, 1], F32, tag="epsb")
        nc.vector.memset(epsb[:], LN_EPS)
        pwt_sb = const_pool.tile([128, 2, DIM], BF16, tag="pwt")
        nc.scalar.dma_start(pwt_sb[:], pwt_d[:])
        relsl_sb = const_pool.tile([128, N_TILES, DIM], BF16, tag="relsl")
        nc.scalar.dma_start(relsl_sb[:], relsl_d[:])

        # --- the priority DMA stream on the sync ring ---
        # X and W0 interleaved in PE consumption order, then W1..W9.
        x_sb = const_pool.tile([128, S_TILES, DIM], FP8, tag="x")
        w_sb = const_pool.tile([128, N_TILES, S_TILES, TILE], FP8,
                               name="w_sb", tag="w_sb")
        w_t = [w_sb[:, t] for t in range(N_TILES)]
        for i in range(len(CHUNKS0) - 1):
            lo, hi = CHUNKS0[i], CHUNKS0[i + 1]
            nc.sync.dma_start(x_sb[:, lo:hi, :], x_d[:, lo * DIM:hi * DIM])
            nc.sync.dma_start(w_t[0][:, lo:hi, :],
                              w_d[0, :, lo * TILE:hi * TILE])
        for t in range(1, N_TILES):
            ch = CHUNKS_LAST if t == N_TILES - 1 else CHUNKS
            for i in range(len(ch) - 1):
                lo, hi = ch[i], ch[i + 1]
                nc.sync.dma_start(w_t[t][:, lo:hi, :],
                                  w_d[t, :, lo * TILE:hi * TILE])

        def epilogue(t, agg_ps):
            # LN -> transpose -> @ proj_w.T -> residual
            agg = ep_pool.tile([128, DIM], F32, tag="agg_sb")
            rowsum = ep_pool.tile([128, 1], F32, tag="rowsum")
            nc.scalar.activation(agg[:], agg_ps[:], AF.Copy,
                                 accum_out=rowsum[:])
            mean = ep_pool.tile([128, 1], F32, tag="mean")
            nc.scalar.mul(mean[:], rowsum[:], 1.0 / DIM)
            cent = ep_pool.tile([128, DIM], F32, tag="cent")
            nc.vector.tensor_scalar(cent[:], agg[:], mean[:], None,
                                    AluOpType.subtract)
            sq = ep_pool.tile([128, DIM], F32, tag="sq")
            sumsq = ep_pool.tile([128, 1], F32, tag="sumsq")
            nc.scalar.activation(sq[:], cent[:], AF.Square,
                                 accum_out=sumsq[:])
            std = ep_pool.tile([128, 1], F32, tag="std")
            nc.scalar.activation(std[:], sumsq[:], AF.Sqrt, bias=epsb[:],
                                 scale=1.0 / DIM)
            rstd = ep_pool.tile([128, 1], F32, tag="rstd")
            nc.vector.reciprocal(rstd[:], std[:])
            ln = ep_pool.tile([128, DIM], BF16, tag="ln")
            nc.vector.tensor_scalar(ln[:], cent[:], rstd[:], None,
                                    AluOpType.mult)

            y_ps = ps_y.tile([128, DIM], F32, tag="y")
            for k in range(2):
                tr_ps = ps_tr.tile([128, 128], BF16, tag="tr")
                nc.tensor.transpose(tr_ps[:], ln[:, k * 128:(k + 1) * 128],
                                    ident[:])
                lnT = ep_pool.tile([128, 128], BF16, tag="lnT")
                nc.scalar.copy(lnT[:], tr_ps[:])
                nc.tensor.matmul(y_ps[:], lnT[:], pwt_sb[:, k, :],
                                 start=(k == 0), stop=(k == 1))

            delta = ep_pool.tile([128, DIM], F32, tag="delta")
            nc.vector.tensor_scalar(delta[:], y_ps[:], ALPHA, None,
                                    AluOpType.mult)
            out_t = ep_pool.tile([128, DIM], BF16, tag="out")
            nc.vector.tensor_tensor(out_t[:], delta[:],
                                    relsl_sb[:, t, :], AluOpType.add)
            nc.sync.dma_start(out_d[:, t * DIM:(t + 1) * DIM], out_t[:])

        # software-pipelined: tile t's block MMs are emitted before tile
        # t-1's epilogue so the PE never stalls on the previous LN chain
        pending = []
        for t in range(N_TILES):
            agg_ps = ps_agg.tile([128, DIM], F32, tag="agg")
            for j in range(S_PAIRS):
                nc.tensor.matmul(agg_ps[:], w_t[t][:, 2 * j:2 * j + 2, :],
                                 x_sb[:, 2 * j:2 * j + 2, :],
                                 start=(j == 0), stop=False,
                                 perf_mode=mybir.MatmulPerfMode.DoubleRow)
            # odd tail tile (no DoubleRow partner)
            nc.tensor.matmul(agg_ps[:], w_t[t][:, S_TILES - 1, :],
                             x_sb[:, S_TILES - 1, :],
                             start=False, stop=True)
            pending.append((t, agg_ps))
            if len(pending) > 2:
                epilogue(*pending.pop(0))
        for p in pending:
            epilogue(*p)

    nc.compile()
    return nc


def _prep(rel_embed, rel_edge_index, rel_edge_weight, proj_w):
    """Host-side sharding/layout: scatter edges into dense per-(dst tile,
    src tile) weight blocks; lay out rel_embed for SBUF residency."""
    src = np.asarray(rel_edge_index[0], dtype=np.int64)
    dst = np.asarray(rel_edge_index[1], dtype=np.int64)
    w = np.asarray(rel_edge_weight, dtype=np.float32)
    rel = np.asarray(rel_embed, dtype=np.float32)
    pw = np.asarray(proj_w, dtype=np.float32)

    core = dst // NODES_PER_CORE
    drel = dst - core * NODES_PER_CORE
    t = drel // TILE
    d = drel % TILE
    s = src // TILE
    p = src % TILE
    # flat index inside one core's [N_TILES, S_TILES, 128, 128] block array
    flat = ((t * S_TILES + s) * TILE + p) * TILE + d
    blk_sz = N_TILES * S_TILES * TILE * TILE

    w_dev = np.empty((N_CORES, N_TILES, 128, S_TILES * TILE),
                     dtype=ml_dtypes.float8_e4m3)
    for c in range(N_CORES):
        m = core == c
        wc = np.bincount(flat[m], weights=w[m], minlength=blk_sz)
        wc = wc.reshape(N_TILES, S_TILES, TILE, TILE).astype(np.float32)
        # -> [t, p(src), s*128+d(dst)] so the SBUF tile is partition=src
        w_dev[c] = wc.transpose(0, 2, 1, 3).reshape(
            N_TILES, 128, S_TILES * TILE)

    rel8 = rel.astype(ml_dtypes.float8_e4m3)
    rel8_pad = np.zeros((S_TILES * TILE, DIM), dtype=ml_dtypes.float8_e4m3)
    rel8_pad[:N_NODES] = rel8
    x_dev = np.ascontiguousarray(
        rel8_pad.reshape(S_TILES, TILE, DIM).transpose(1, 0, 2).reshape(
            128, S_TILES * DIM))

    rel16 = rel.astype(ml_dtypes.bfloat16)
    relsl = np.zeros((N_CORES, 128, N_TILES, DIM), dtype=ml_dtypes.bfloat16)
    for c in range(N_CORES):
        sl = np.zeros((OUT_ROWS, DIM), dtype=ml_dtypes.bfloat16)
        sl[:NODES_PER_CORE] = rel16[c * NODES_PER_CORE:(c + 1) * NODES_PER_CORE]
        relsl[c] = sl.reshape(N_TILES, 128, DIM).transpose(1, 0, 2)
    relsl = relsl.reshape(N_CORES, 128, N_TILES * DIM)

    pwt = pw.T.astype(ml_dtypes.bfloat16)  # [f, o]
    pwt_dev = np.ascontiguousarray(
        pwt.reshape(2, 128, DIM).transpose(1, 0, 2).reshape(128, 2 * DIM))

    in_maps = []
    for c in range(N_CORES):
        in_maps.append({
            "x": x_dev,
            "wblk": w_dev[c],
            "relslice": np.ascontiguousarray(relsl[c]),
            "projwT": pwt_dev,
        })
    return in_maps


def kernel(rel_embed, rel_edge_index, rel_edge_weight, proj_w,
           _trace=False):
    in_maps = _prep(rel_embed, rel_edge_index, rel_edge_weight, proj_w)
    nc = _CACHE.get("nc")
    if nc is None:
        nc = _build()
        _CACHE["nc"] = nc
    res = run_bass_kernel_spmd(nc, in_maps, core_ids=list(range(N_CORES)),
                               trace=_trace)
    parts = []
    for c in range(N_CORES):
        o = np.asarray(res.results[c]["out"]).reshape(128, N_TILES, DIM)
        o = o.transpose(1, 0, 2).reshape(OUT_ROWS, DIM)[:NODES_PER_CORE]
        parts.append(o)
    out = np.concatenate(parts, axis=0)
    if _trace:
        kernel.last_results = res
    return out.astype(np.float32)


# revision 8
# speedup vs baseline: 1.0280x; 1.0280x over previous
"""DualRelGCN message-passing kernel for 8 TRN2 NeuronCores.

Strategy (destination-sharded, collective-free, distinct-src folding):
  - LayerNorm is invariant to positive per-row scaling, so LN(agg/denom) ==
    LN(agg): the denominator drops out of the computation entirely.
  - Shard edges by dst range: core c owns nodes [1250c, 1250(c+1)) and
    computes its 1250 output rows completely locally -> no collectives.
  - Per dst tile (128 dsts) only ~3.1k of the 10k src nodes have an edge in,
    so the host compacts the active src rows into <=28 "fold" tiles of 128:
    X~[fold] = gathered rel_embed rows (fp8), W~[fold] = [128 src x 128 dst]
    dense weight block over the compacted rows.  agg_t = sum_f W~_f.T X~_f.
    vs. the all-src dense formulation this is ~3x fewer PE block-matmuls
    (the PE runs DoubleRow fp8 at ~256 cycles per 2-tile pair on this hw)
    and ~15% fewer HBM bytes.
  - X~ row i and its weight row ride adjacent in one interleaved stream
    ([128, fold, 256+128] fp8), DMA'd in exact PE consumption order, fully
    SBUF-resident -> single full-bandwidth burst, no recycle stalls.
  - Epilogue per dst tile: LN on ACT/DVE, PE transpose, y = ln @ proj_w.T,
    out = rel + 0.1*y in bf16 (host upcasts to fp32).
"""

import sys

for _p in ("/opt/trn_rl_repo",):
    if _p not in sys.path:
        sys.path.insert(0, _p)

from contextlib import ExitStack

import numpy as np
import ml_dtypes

import concourse.bacc as bacc
import concourse.mybir as mybir
from concourse.alu_op_type import AluOpType
from concourse.tile import TileContext
from concourse.bass_utils import run_bass_kernel_spmd

F32 = mybir.dt.float32
BF16 = mybir.dt.bfloat16
FP8 = mybir.dt.float8e4
AF = mybir.ActivationFunctionType

N_NODES = 10000
DIM = 256
N_CORES = 8
NODES_PER_CORE = N_NODES // N_CORES  # 1250
TILE = 128
N_TILES = -(-NODES_PER_CORE // TILE)  # 10 dst tiles per core
OUT_ROWS = N_TILES * TILE  # 1280
F_MAX = 28  # fold tiles per dst tile (max observed 27; zero-padded)
F_PAIRS = F_MAX // 2
REC = DIM + TILE  # 384 interleaved bytes per fold row: X~ row | W~ row
ALPHA = 0.1
LN_EPS = 1e-5
# chunk boundaries (fold units): fine-grained for tile 0 so the PE starts
# early; a small tail chunk on the last tile shortens the drain
CH0 = [0, 4, 12, 20, F_MAX]
CH = [0, 14, F_MAX]
CH_LAST = [0, 14, 22, F_MAX]

_CACHE: dict = {}


def _build():
    nc = bacc.Bacc("TRN2", target_bir_lowering=False, debug=False,
                   num_devices=N_CORES)

    xw_d = nc.dram_tensor("xw", [N_TILES, 128, F_MAX * REC], FP8,
                          kind="ExternalInput")
    ident_d = nc.dram_tensor("ident", [128, 128], BF16, kind="ExternalInput")
    relsl_d = nc.dram_tensor("relslice", [128, N_TILES * DIM], BF16,
                             kind="ExternalInput")
    pwt_d = nc.dram_tensor("projwT", [128, 2 * DIM], BF16,
                           kind="ExternalInput")
    out_d = nc.dram_tensor("out", [128, N_TILES * DIM], BF16,
                           kind="ExternalOutput")

    with TileContext(nc) as tc, ExitStack() as es:
        const_pool = es.enter_context(tc.tile_pool(name="const", bufs=1))
        ep_pool = es.enter_context(tc.tile_pool(name="ep", bufs=2))
        ps_agg = es.enter_context(tc.tile_pool(name="ps_agg", bufs=3,
                                               space="PSUM"))
        ps_tr = es.enter_context(tc.tile_pool(name="ps_tr", bufs=3,
                                              space="PSUM"))
        ps_y = es.enter_context(tc.tile_pool(name="ps_y", bufs=2,
                                             space="PSUM"))

        # --- small resident inputs (scalar ring) ---
        epsb = const_pool.tile([128, 1], F32, tag="epsb")
        nc.vector.memset(epsb[:], LN_EPS)
        ident = const_pool.tile([128, 128], BF16, tag="ident")
        nc.scalar.dma_start(ident[:], ident_d[:])
        pwt_sb = const_pool.tile([128, 2, DIM], BF16, tag="pwt")
        nc.scalar.dma_start(pwt_sb[:], pwt_d[:])
        relsl_sb = const_pool.tile([128, N_TILES, DIM], BF16, tag="relsl")
        nc.scalar.dma_start(relsl_sb[:], relsl_d[:])

        # --- the fold stream, in PE consumption order (sync ring) ---
        xw_sb = const_pool.tile([128, N_TILES, F_MAX, REC], FP8, tag="xw")
        for t in range(N_TILES):
            ch = CH0 if t == 0 else (CH_LAST if t == N_TILES - 1 else CH)
            for i in range(len(ch) - 1):
                lo, hi = ch[i], ch[i + 1]
                nc.sync.dma_start(xw_sb[:, t, lo:hi, :],
                                  xw_d[t, :, lo * REC:hi * REC])

        def epilogue(t, agg_ps):
            # LN -> transpose -> @ proj_w.T -> residual
            agg = ep_pool.tile([128, DIM], F32, tag="agg_sb")
            rowsum = ep_pool.tile([128, 1], F32, tag="rowsum")
            nc.scalar.activation(agg[:], agg_ps[:], AF.Copy,
                                 accum_out=rowsum[:])
            mean = ep_pool.tile([128, 1], F32, tag="mean")
            nc.scalar.mul(mean[:], rowsum[:], 1.0 / DIM)
            cent = ep_pool.tile([128, DIM], F32, tag="cent")
            nc.vector.tensor_scalar(cent[:], agg[:], mean[:], None,
                                    AluOpType.subtract)
            sq = ep_pool.tile([128, DIM], F32, tag="sq")
            sumsq = ep_pool.tile([128, 1], F32, tag="sumsq")
            nc.scalar.activation(sq[:], cent[:], AF.Square,
                                 accum_out=sumsq[:])
            std = ep_pool.tile([128, 1], F32, tag="std")
            nc.scalar.activation(std[:], sumsq[:], AF.Sqrt, bias=epsb[:],
                                 scale=1.0 / DIM)
            rstd = ep_pool.tile([128, 1], F32, tag="rstd")
            nc.vector.reciprocal(rstd[:], std[:])
            ln = ep_pool.tile([128, DIM], BF16, tag="ln")
            nc.vector.tensor_scalar(ln[:], cent[:], rstd[:], None,
                                    AluOpType.mult)

            y_ps = ps_y.tile([128, DIM], F32, tag="y")
            for k in range(2):
                tr_ps = ps_tr.tile([128, 128], BF16, tag="tr")
                nc.tensor.transpose(tr_ps[:], ln[:, k * 128:(k + 1) * 128],
                                    ident[:])
                lnT = ep_pool.tile([128, 128], BF16, tag="lnT")
                nc.scalar.copy(lnT[:], tr_ps[:])
                nc.tensor.matmul(y_ps[:], lnT[:], pwt_sb[:, k, :],
                                 start=(k == 0), stop=(k == 1))

            delta = ep_pool.tile([128, DIM], F32, tag="delta")
            nc.vector.tensor_scalar(delta[:], y_ps[:], ALPHA, None,
                                    AluOpType.mult)
            out_t = ep_pool.tile([128, DIM], BF16, tag="out")
            nc.vector.tensor_tensor(out_t[:], delta[:],
                                    relsl_sb[:, t, :], AluOpType.add)
            nc.sync.dma_start(out_d[:, t * DIM:(t + 1) * DIM], out_t[:])

        # software-pipelined: tile t's block MMs are emitted before tile
        # t-1's epilogue so the PE never stalls on the previous LN chain
        pending = []
        for t in range(N_TILES):
            agg_ps = ps_agg.tile([128, DIM], F32, tag="agg")
            for p in range(F_PAIRS):
                nc.tensor.matmul(agg_ps[:],
                                 xw_sb[:, t, 2 * p:2 * p + 2, DIM:REC],
                                 xw_sb[:, t, 2 * p:2 * p + 2, 0:DIM],
                                 start=(p == 0), stop=(p == F_PAIRS - 1),
                                 perf_mode=mybir.MatmulPerfMode.DoubleRow)
            pending.append((t, agg_ps))
            if len(pending) > 1:
                epilogue(*pending.pop(0))
        for p in pending:
            epilogue(*p)

    nc.compile()
    return nc


def _prep(rel_embed, rel_edge_index, rel_edge_weight, proj_w):
    """Host-side sharding/layout: per (core, dst tile), compact the distinct
    src rows into fold tiles and interleave gathered X~ rows with their
    W~ weight rows in one stream."""
    src = np.asarray(rel_edge_index[0], dtype=np.int64)
    dst = np.asarray(rel_edge_index[1], dtype=np.int64)
    w = np.asarray(rel_edge_weight, dtype=np.float32)
    rel = np.asarray(rel_embed, dtype=np.float32)
    pw = np.asarray(proj_w, dtype=np.float32)

    rel8 = rel.astype(ml_dtypes.float8_e4m3)
    core = dst // NODES_PER_CORE
    drel = dst - core * NODES_PER_CORE
    tt = drel // TILE
    dd = drel % TILE

    xw_dev = np.zeros((N_CORES, N_TILES, F_MAX * TILE, REC),
                      dtype=ml_dtypes.float8_e4m3)
    order = np.lexsort((src, tt, core))
    so, to, co, do_, wo = (src[order], tt[order], core[order], dd[order],
                           w[order])
    grp = co * N_TILES + to
    starts = np.searchsorted(grp, np.arange(N_CORES * N_TILES))
    ends = np.append(starts[1:], len(grp))
    for c in range(N_CORES):
        for t in range(N_TILES):
            a, b = starts[c * N_TILES + t], ends[c * N_TILES + t]
            s_ct, d_ct, w_ct = so[a:b], do_[a:b], wo[a:b]
            uniq = np.unique(s_ct)
            assert len(uniq) <= F_MAX * TILE, len(uniq)
            slot = np.searchsorted(uniq, s_ct)
            wblk = np.bincount(slot * TILE + d_ct, weights=w_ct,
                               minlength=F_MAX * TILE * TILE)
            blk = xw_dev[c, t]
            blk[:, DIM:] = wblk.reshape(F_MAX * TILE, TILE).astype(np.float32)
            blk[:len(uniq), :DIM] = rel8[uniq]
    # -> [t, partition(i), fold*REC]: row i of fold f holds src slot f*128+i
    xw_dev = np.ascontiguousarray(
        xw_dev.reshape(N_CORES, N_TILES, F_MAX, TILE, REC)
        .transpose(0, 1, 3, 2, 4).reshape(N_CORES, N_TILES, 128,
                                          F_MAX * REC))

    rel16 = rel.astype(ml_dtypes.bfloat16)
    relsl = np.zeros((N_CORES, 128, N_TILES * DIM), dtype=ml_dtypes.bfloat16)
    for c in range(N_CORES):
        sl = np.zeros((OUT_ROWS, DIM), dtype=ml_dtypes.bfloat16)
        sl[:NODES_PER_CORE] = rel16[c * NODES_PER_CORE:
                                    (c + 1) * NODES_PER_CORE]
        relsl[c] = sl.reshape(N_TILES, 128, DIM).transpose(1, 0, 2).reshape(
            128, N_TILES * DIM)

    pwt = pw.T.astype(ml_dtypes.bfloat16)  # [f, o]
    pwt_dev = np.ascontiguousarray(
        pwt.reshape(2, 128, DIM).transpose(1, 0, 2).reshape(128, 2 * DIM))
    ident_dev = np.eye(128, dtype=ml_dtypes.bfloat16)

    in_maps = []
    for c in range(N_CORES):
        in_maps.append({
            "xw": xw_dev[c],
            "ident": ident_dev,
            "relslice": np.ascontiguousarray(relsl[c]),
            "projwT": pwt_dev,
        })
    return in_maps


def kernel(rel_embed, rel_edge_index, rel_edge_weight, proj_w,
           _trace=False):
    in_maps = _prep(rel_embed, rel_edge_index, rel_edge_weight, proj_w)
    nc = _CACHE.get("nc")
    if nc is None:
        nc = _build()
        _CACHE["nc"] = nc
    res = run_bass_kernel_spmd(nc, in_maps, core_ids=list(range(N_CORES)),
                               trace=_trace)
    parts = []
    for c in range(N_CORES):
        o = np.asarray(res.results[c]["out"]).reshape(128, N_TILES, DIM)
        o = o.transpose(1, 0, 2).reshape(OUT_ROWS, DIM)[:NODES_PER_CORE]
        parts.append(o)
    out = np.concatenate(parts, axis=0)
    if _trace:
        kernel.last_results = res
    return out.astype(np.float32)


# revision 9
# speedup vs baseline: 1.4046x; 1.3663x over previous
"""DualRelGCN message-passing kernel for 8 TRN2 NeuronCores.

Strategy (destination-sharded, collective-free, distinct-src folding):
  - LayerNorm is invariant to positive per-row scaling, so LN(agg/denom) ==
    LN(agg): the denominator drops out of the computation entirely.
  - Shard edges by dst range: core c owns nodes [1250c, 1250(c+1)) and
    computes its 1250 output rows completely locally -> no collectives.
  - Per dst tile (128 dsts) only ~3.1k of the 10k src nodes have an edge in,
    so the host compacts the active src rows into <=28 "fold" tiles of 128:
    X~[fold] = gathered rel_embed rows (fp8), W~[fold] = [128 src x 128 dst]
    dense weight block over the compacted rows.  agg_t = sum_f W~_f.T X~_f.
    vs. the all-src dense formulation this is ~3x fewer PE block-matmuls
    (the PE runs DoubleRow fp8 at ~256 cycles per 2-tile pair on this hw)
    and ~15% fewer HBM bytes.
  - X~ row i and its weight row ride adjacent in one interleaved stream
    ([128, fold, 256+128] fp8), DMA'd in exact PE consumption order, fully
    SBUF-resident -> single full-bandwidth burst, no recycle stalls.
  - Epilogue per dst tile: LN on ACT/DVE, PE transpose, y = ln @ proj_w.T,
    out = rel + 0.1*y in bf16 (host upcasts to fp32).
"""

import sys

for _p in ("/opt/trn_rl_repo",):
    if _p not in sys.path:
        sys.path.insert(0, _p)

from contextlib import ExitStack

import numpy as np
import ml_dtypes

import concourse.bacc as bacc
import concourse.mybir as mybir
from concourse.alu_op_type import AluOpType
from concourse.tile import TileContext
from concourse.bass_utils import run_bass_kernel_spmd

F32 = mybir.dt.float32
BF16 = mybir.dt.bfloat16
FP8 = mybir.dt.float8e4
AF = mybir.ActivationFunctionType

N_NODES = 10000
DIM = 256
N_CORES = 8
NODES_PER_CORE = N_NODES // N_CORES  # 1250
TILE = 128
N_TILES = -(-NODES_PER_CORE // TILE)  # 10 dst tiles per core
OUT_ROWS = N_TILES * TILE  # 1280
F_MAX = 28  # fold tiles per dst tile (max observed 27; zero-padded)
F_PAIRS = F_MAX // 2
REC = DIM + TILE  # 384 interleaved bytes per fold row: X~ row | W~ row
ALPHA = 0.1
LN_EPS = 1e-5
# chunk boundaries (fold units): fine-grained for tile 0 so the PE starts
# early; a small tail chunk on the last tile shortens the drain
CH0 = [0, 4, 12, 20, F_MAX]
CH = [0, 14, F_MAX]
CH_LAST = [0, 14, 22, F_MAX]

_CACHE: dict = {}


def _build():
    nc = bacc.Bacc("TRN2", target_bir_lowering=False, debug=False,
                   num_devices=N_CORES)

    xt_d = nc.dram_tensor("xt", [N_TILES, 128, F_MAX * DIM], FP8,
                          kind="ExternalInput")
    wt_d = nc.dram_tensor("wt", [N_TILES, 128, F_MAX * TILE], FP8,
                          kind="ExternalInput")
    ident_d = nc.dram_tensor("ident", [128, 128], BF16, kind="ExternalInput")
    relsl_d = nc.dram_tensor("relslice", [128, N_TILES * DIM], BF16,
                             kind="ExternalInput")
    pwt_d = nc.dram_tensor("projwT", [128, 2 * DIM], BF16,
                           kind="ExternalInput")
    out_d = nc.dram_tensor("out", [128, N_TILES * DIM], BF16,
                           kind="ExternalOutput")

    with TileContext(nc) as tc, ExitStack() as es:
        const_pool = es.enter_context(tc.tile_pool(name="const", bufs=1))
        ep_pool = es.enter_context(tc.tile_pool(name="ep", bufs=2))
        ps_agg = es.enter_context(tc.tile_pool(name="ps_agg", bufs=3,
                                               space="PSUM"))
        ps_tr = es.enter_context(tc.tile_pool(name="ps_tr", bufs=3,
                                              space="PSUM"))
        ps_y = es.enter_context(tc.tile_pool(name="ps_y", bufs=2,
                                             space="PSUM"))

        # --- small resident inputs (scalar ring) ---
        epsb = const_pool.tile([128, 1], F32, tag="epsb")
        nc.vector.memset(epsb[:], LN_EPS)
        ident = const_pool.tile([128, 128], BF16, tag="ident")
        nc.scalar.dma_start(ident[:], ident_d[:])
        pwt_sb = const_pool.tile([128, 2, DIM], BF16, tag="pwt")
        nc.scalar.dma_start(pwt_sb[:], pwt_d[:])
        relsl_sb = const_pool.tile([128, N_TILES, DIM], BF16, tag="relsl")
        nc.scalar.dma_start(relsl_sb[:], relsl_d[:])

        # --- the fold stream, in PE consumption order (sync ring) ---
        xt_sb = const_pool.tile([128, N_TILES, F_MAX, DIM], FP8, tag="xt")
        wt_sb = const_pool.tile([128, N_TILES, F_MAX, TILE], FP8, tag="wt")
        for t in range(N_TILES):
            ch = CH0 if t == 0 else (CH_LAST if t == N_TILES - 1 else CH)
            for i in range(len(ch) - 1):
                lo, hi = ch[i], ch[i + 1]
                nc.sync.dma_start(xt_sb[:, t, lo:hi, :],
                                  xt_d[t, :, lo * DIM:hi * DIM])
                nc.sync.dma_start(wt_sb[:, t, lo:hi, :],
                                  wt_d[t, :, lo * TILE:hi * TILE])

        def epilogue(t, agg_ps):
            # LN (bn_stats one-pass mean/var) -> transpose -> @ proj_w.T
            stats = ep_pool.tile([128, 6], F32, tag="stats")
            nc.vector.bn_stats(stats[:], agg_ps[:])
            mv = ep_pool.tile([128, 2], F32, tag="mv")
            nc.vector.bn_aggr(mv[:], stats[:])
            std = ep_pool.tile([128, 1], F32, tag="std")
            nc.scalar.activation(std[:], mv[:, 1:2], AF.Sqrt, bias=epsb[:])
            rstd = ep_pool.tile([128, 1], F32, tag="rstd")
            nc.vector.reciprocal(rstd[:], std[:])
            ln = ep_pool.tile([128, DIM], BF16, tag="ln")
            nc.vector.tensor_scalar(ln[:], agg_ps[:], mv[:, 0:1], rstd[:],
                                    AluOpType.subtract, AluOpType.mult)

            y_ps = ps_y.tile([128, DIM], F32, tag="y")
            for k in range(2):
                tr_ps = ps_tr.tile([128, 128], BF16, tag="tr")
                nc.tensor.transpose(tr_ps[:], ln[:, k * 128:(k + 1) * 128],
                                    ident[:])
                lnT = ep_pool.tile([128, 128], BF16, tag="lnT")
                nc.scalar.copy(lnT[:], tr_ps[:])
                nc.tensor.matmul(y_ps[:], lnT[:], pwt_sb[:, k, :],
                                 start=(k == 0), stop=(k == 1))

            delta = ep_pool.tile([128, DIM], F32, tag="delta")
            nc.vector.tensor_scalar(delta[:], y_ps[:], ALPHA, None,
                                    AluOpType.mult)
            out_t = ep_pool.tile([128, DIM], BF16, tag="out")
            nc.vector.tensor_tensor(out_t[:], delta[:],
                                    relsl_sb[:, t, :], AluOpType.add)
            nc.sync.dma_start(out_d[:, t * DIM:(t + 1) * DIM], out_t[:])

        # software-pipelined: tile t's block MMs are emitted before tile
        # t-1's epilogue so the PE never stalls on the previous LN chain
        pending = []
        for t in range(N_TILES):
            agg_ps = ps_agg.tile([128, DIM], F32, tag="agg")
            for p in range(F_PAIRS):
                nc.tensor.matmul(agg_ps[:],
                                 wt_sb[:, t, 2 * p:2 * p + 2, :],
                                 xt_sb[:, t, 2 * p:2 * p + 2, :],
                                 start=(p == 0), stop=(p == F_PAIRS - 1),
                                 perf_mode=mybir.MatmulPerfMode.DoubleRow)
            pending.append((t, agg_ps))
            if len(pending) > 2:
                epilogue(*pending.pop(0))
        for p in pending:
            epilogue(*p)

    nc.compile()
    return nc


def _prep(rel_embed, rel_edge_index, rel_edge_weight, proj_w):
    """Host-side sharding/layout: per (core, dst tile), compact the distinct
    src rows into fold tiles and interleave gathered X~ rows with their
    W~ weight rows in one stream."""
    src = np.asarray(rel_edge_index[0], dtype=np.int64)
    dst = np.asarray(rel_edge_index[1], dtype=np.int64)
    w = np.asarray(rel_edge_weight, dtype=np.float32)
    rel = np.asarray(rel_embed, dtype=np.float32)
    pw = np.asarray(proj_w, dtype=np.float32)

    rel8 = rel.astype(ml_dtypes.float8_e4m3)
    core = dst // NODES_PER_CORE
    drel = dst - core * NODES_PER_CORE
    tt = drel // TILE
    dd = drel % TILE

    xt_dev = np.zeros((N_CORES, N_TILES, F_MAX * TILE, DIM),
                      dtype=ml_dtypes.float8_e4m3)
    wt_dev = np.zeros((N_CORES, N_TILES, F_MAX * TILE, TILE),
                      dtype=ml_dtypes.float8_e4m3)
    order = np.lexsort((src, tt, core))
    so, to, co, do_, wo = (src[order], tt[order], core[order], dd[order],
                           w[order])
    grp = co * N_TILES + to
    starts = np.searchsorted(grp, np.arange(N_CORES * N_TILES))
    ends = np.append(starts[1:], len(grp))
    for c in range(N_CORES):
        for t in range(N_TILES):
            a, b = starts[c * N_TILES + t], ends[c * N_TILES + t]
            s_ct, d_ct, w_ct = so[a:b], do_[a:b], wo[a:b]
            uniq = np.unique(s_ct)
            assert len(uniq) <= F_MAX * TILE, len(uniq)
            slot = np.searchsorted(uniq, s_ct)
            wblk = np.bincount(slot * TILE + d_ct, weights=w_ct,
                               minlength=F_MAX * TILE * TILE)
            wt_dev[c, t] = wblk.reshape(F_MAX * TILE, TILE).astype(
                np.float32)
            xt_dev[c, t, :len(uniq)] = rel8[uniq]
    # -> [t, partition(i), fold*d]: row i of fold f holds src slot f*128+i
    xt_dev = np.ascontiguousarray(
        xt_dev.reshape(N_CORES, N_TILES, F_MAX, TILE, DIM)
        .transpose(0, 1, 3, 2, 4).reshape(N_CORES, N_TILES, 128,
                                          F_MAX * DIM))
    wt_dev = np.ascontiguousarray(
        wt_dev.reshape(N_CORES, N_TILES, F_MAX, TILE, TILE)
        .transpose(0, 1, 3, 2, 4).reshape(N_CORES, N_TILES, 128,
                                          F_MAX * TILE))

    rel16 = rel.astype(ml_dtypes.bfloat16)
    relsl = np.zeros((N_CORES, 128, N_TILES * DIM), dtype=ml_dtypes.bfloat16)
    for c in range(N_CORES):
        sl = np.zeros((OUT_ROWS, DIM), dtype=ml_dtypes.bfloat16)
        sl[:NODES_PER_CORE] = rel16[c * NODES_PER_CORE:
                                    (c + 1) * NODES_PER_CORE]
        relsl[c] = sl.reshape(N_TILES, 128, DIM).transpose(1, 0, 2).reshape(
            128, N_TILES * DIM)

    pwt = pw.T.astype(ml_dtypes.bfloat16)  # [f, o]
    pwt_dev = np.ascontiguousarray(
        pwt.reshape(2, 128, DIM).transpose(1, 0, 2).reshape(128, 2 * DIM))
    ident_dev = np.eye(128, dtype=ml_dtypes.bfloat16)

    in_maps = []
    for c in range(N_CORES):
        in_maps.append({
            "xt": xt_dev[c],
            "wt": wt_dev[c],
            "ident": ident_dev,
            "relslice": np.ascontiguousarray(relsl[c]),
            "projwT": pwt_dev,
        })
    return in_maps


def kernel(rel_embed, rel_edge_index, rel_edge_weight, proj_w,
           _trace=False):
    in_maps = _prep(rel_embed, rel_edge_index, rel_edge_weight, proj_w)
    nc = _CACHE.get("nc")
    if nc is None:
        nc = _build()
        _CACHE["nc"] = nc
    res = run_bass_kernel_spmd(nc, in_maps, core_ids=list(range(N_CORES)),
                               trace=_trace)
    parts = []
    for c in range(N_CORES):
        o = np.asarray(res.results[c]["out"]).reshape(128, N_TILES, DIM)
        o = o.transpose(1, 0, 2).reshape(OUT_ROWS, DIM)[:NODES_PER_CORE]
        parts.append(o)
    out = np.concatenate(parts, axis=0)
    if _trace:
        kernel.last_results = res
    return out.astype(np.float32)
